# Initial kernel scaffold
#
"""Trainium2 Bass kernel for ModalEnseModel (aware-score fusion + modality concat).

Reference op (per batch item b):
    out[b] = concat([ concat([vis[b,:, :5], vis[b,:,5:] * s[b]], axis=-1),
                      lwir[b] ], axis=0)          # [2N, C]

Full shapes: vis/lwir [32, 25200, 85] f32, aware [32, 1] f32 -> out [32, 50400, 85].

Strategy: pure data parallel over batch -- 4 images per NeuronCore x 8 cores.
Per core:
  * lwir half of the output is one straight 34MB DRAM->DRAM DMA
    (no compute), issued on the GPSIMD/SWDGE queue.
  * visible half streams through SBUF in ~2.1MB tiles [128, 50, 85]
    (50 rows of one image per partition); a single in-place
    tensor_scalar multiply on the [:, :, 5:] slice applies the per-image
    scale (broadcast to [128,1] by a tiny DMA), then the whole tile is
    DMA'd to the output. Loads issue on the SP HWDGE ring, stores on the
    ACT HWDGE ring so a store's wait-on-DVE never head-of-line-blocks
    later loads (3 independent DMA issue streams total; ~8% faster than
    everything on nc.sync).
Outputs are two DRAM tensors (visible half / lwir half of the anchor
axis), concatenated host-side in the gather: a single fused output made
neuronxcc reject the store DMAs ("Too many sync wait commands").

All traffic is the intrinsic 137MB/core: read vis (34.3MB) + read lwir
(34.3MB) + write both halves (68.5MB) -- the mathematical minimum.
Nominal roofline 382us at the ~358GB/s per-NC HBM limit. Measured
(reps-slope method, see bench.py): ~410-430us/iteration, and a
pure-DMA ceiling probe with identical traffic but no SBUF/compute
measures the same ~412-416us -- i.e. the kernel sits at the empirical
HBM hardware ceiling and the multiply is fully hidden under DMA.
"""

import numpy as np

from concourse import bacc, bass, mybir
from concourse.bass_utils import run_bass_kernel_spmd
from concourse.tile import TileContext

F32 = mybir.dt.float32

B, N, C = 32, 25200, 85
NCORES = 8
PER = B // NCORES  # images per core

_BUILD_CACHE: dict = {}


def build_nc(per=PER, n=N, c=C, n_scaled_from=5, rows_per_part=50, bufs=8,
             reps=1, lwir_whole=True, store_eng="scalar", lwir_eng="gpsimd",
             sc_eng="gpsimd", alt_outputs=False, interleave_ls=False):
    """Build the single-core Bass program (SPMD: same program on all cores).

    reps>1 repeats the whole body (for benchmarking: amortizes dispatch
    noise); the op is idempotent so results are unchanged.
    """
    nc = bacc.Bacc()
    vis = nc.dram_tensor("vis", [per, n, c], F32, kind="ExternalInput")
    lwir = nc.dram_tensor("lwir", [per, n, c], F32, kind="ExternalInput")
    aware = nc.dram_tensor("aware", [per], F32, kind="ExternalInput")
    # Two outputs (visible half / lwir half of the anchor axis): keeps the
    # lwir passthrough DMAs free of WAW deps against the visible stores --
    # a single fused output tensor made neuronxcc reject the store DMAs
    # ("Too many sync wait commands"). The host gather concatenates.
    out_v = nc.dram_tensor("out_v", [per, n, c], F32, kind="ExternalOutput")
    out_l = nc.dram_tensor("out_l", [per, n, c], F32, kind="ExternalOutput")
    if alt_outputs:
        # bench-only: scratch second output set so odd reps have no WAW
        # deps against even reps (probes whether cross-rep WAW, not HBM,
        # limits the steady-state slope measurement)
        out_v2 = nc.dram_tensor("out_v2", [per, n, c], F32)
        out_l2 = nc.dram_tensor("out_l2", [per, n, c], F32)

    tile_rows = 128 * rows_per_part
    store_q = getattr(nc, store_eng)
    lwir_q = getattr(nc, lwir_eng)
    sc_q = getattr(nc, sc_eng)

    with TileContext(nc) as tc:
        with (
            tc.tile_pool(name="scales", bufs=1) as scpool,
            tc.tile_pool(name="data", bufs=bufs) as pool,
        ):
            sc = scpool.tile([128, per], F32)
            for b in range(per):
                src = aware[b : b + 1].rearrange("(r k) -> r k", r=1)
                sc_q.dma_start(out=sc[:, b : b + 1], in_=src.to_broadcast((128, 1)))

            for _rep in range(reps):
                if alt_outputs and _rep % 2 == 1:
                    o_v, o_l = out_v2, out_l2
                else:
                    o_v, o_l = out_v, out_l
                # lwir passthrough: o_l[b] = lwir[b]
                if lwir_whole:
                    lwir_q.dma_start(out=o_l[:, :, :], in_=lwir[:, :, :])
                else:
                    for b in range(per):
                        lwir_q.dma_start(out=o_l[b, :, :], in_=lwir[b, :, :])

                # visible: scale cols [n_scaled_from:] by s_b through SBUF
                t_idx = 0
                for b in range(per):
                    r = 0
                    while r < n:
                        rows = min(tile_rows, n - r)
                        assert rows % rows_per_part == 0
                        p = rows // rows_per_part
                        tile = pool.tile([p, rows_per_part, c], F32)
                        if interleave_ls:
                            load_q = nc.sync if t_idx % 2 == 0 else nc.scalar
                            st_q = nc.scalar if t_idx % 2 == 0 else nc.sync
                        else:
                            load_q, st_q = nc.sync, store_q
                        t_idx += 1
                        load_q.dma_start(
                            out=tile[:],
                            in_=vis[b, r : r + rows, :].rearrange(
                                "(p k) c -> p k c", p=p
                            ),
                        )
                        nc.vector.tensor_scalar(
                            tile[:, :, n_scaled_from:],
                            tile[:, :, n_scaled_from:],
                            sc[:p, b : b + 1],
                            None,
                            mybir.AluOpType.mult,
                        )
                        st_q.dma_start(
                            out=o_v[b, r : r + rows, :].rearrange(
                                "(p k) c -> p k c", p=p
                            ),
                            in_=tile[:],
                        )
                        r += rows
    nc.compile()
    return nc


def _get_nc():
    if "nc" not in _BUILD_CACHE:
        _BUILD_CACHE["nc"] = build_nc()
    return _BUILD_CACHE["nc"]


def run(inf_out_visible, inf_out_lwir, aware_score, trace=False, **kw):
    nc = _get_nc()
    # Pull everything to host numpy first: harness may hand us jax arrays,
    # and slicing those would dispatch XLA ops on the default (axon) backend.
    vis_np = np.asarray(inf_out_visible, dtype=np.float32)
    lwir_np = np.asarray(inf_out_lwir, dtype=np.float32)
    aw_np = np.asarray(aware_score, dtype=np.float32).reshape(B, -1)[:, 0]
    in_maps = []
    for core in range(NCORES):
        sl = slice(core * PER, (core + 1) * PER)
        in_maps.append(
            {
                "vis": np.ascontiguousarray(vis_np[sl]),
                "lwir": np.ascontiguousarray(lwir_np[sl]),
                "aware": np.ascontiguousarray(aw_np[sl]),
            }
        )
    try:
        res = run_bass_kernel_spmd(
            nc, in_maps, list(range(NCORES)), trace=trace, **kw
        )
    except Exception:
        # one retry: axon tunnel execute failures are transient and the
        # kernel is a pure function of its inputs
        res = run_bass_kernel_spmd(
            nc, in_maps, list(range(NCORES)), trace=trace, **kw
        )
    outs = [
        np.concatenate(
            [res.results[core]["out_v"], res.results[core]["out_l"]], axis=1
        )
        for core in range(NCORES)
    ]
    return np.concatenate(outs, axis=0), res


def kernel(inf_out_visible, inf_out_lwir, aware_score):
    out, _ = run(inf_out_visible, inf_out_lwir, aware_score)
    return out



# revision 1
# speedup vs baseline: 2.0376x; 2.0376x over previous
"""Trainium2 Bass kernel for ModalEnseModel (aware-score fusion + modality concat).

Reference op (per batch item b):
    out[b] = concat([ concat([vis[b,:, :5], vis[b,:,5:] * s[b]], axis=-1),
                      lwir[b] ], axis=0)          # [2N, C]

Full shapes: vis/lwir [32, 25200, 85] f32, aware [32, 1] f32 -> out [32, 50400, 85].

Strategy: pure data parallel over batch -- 4 images per NeuronCore x 8 cores.
Per core:
  * lwir half of the output is one straight 34MB DRAM->DRAM DMA
    (no compute), issued on the GPSIMD/SWDGE queue.
  * visible half streams through SBUF in ~2.1MB tiles [128, 50, 85]
    (50 rows of one image per partition); a single in-place
    tensor_scalar multiply on the [:, :, 5:] slice applies the per-image
    scale (broadcast to [128,1] by a tiny DMA), then the whole tile is
    DMA'd to the output. Loads issue on the SP HWDGE ring, stores on the
    ACT HWDGE ring so a store's wait-on-DVE never head-of-line-blocks
    later loads (3 independent DMA issue streams total; ~8% faster than
    everything on nc.sync).
Outputs are two DRAM tensors (visible half / lwir half of the anchor
axis), concatenated host-side in the gather: a single fused output made
neuronxcc reject the store DMAs ("Too many sync wait commands").

All traffic is the intrinsic 137MB/core: read vis (34.3MB) + read lwir
(34.3MB) + write both halves (68.5MB) -- the mathematical minimum.
Nominal roofline 382us at the ~358GB/s per-NC HBM limit. Measured
(reps-slope method, see bench.py): ~410-430us/iteration, and a
pure-DMA ceiling probe with identical traffic but no SBUF/compute
measures the same ~412-416us -- i.e. the kernel sits at the empirical
HBM hardware ceiling and the multiply is fully hidden under DMA.
"""

import numpy as np

from concourse import bacc, bass, mybir
from concourse.bass_utils import run_bass_kernel_spmd
from concourse.tile import TileContext

F32 = mybir.dt.float32

B, N, C = 32, 25200, 85
NCORES = 8
PER = B // NCORES  # images per core

_BUILD_CACHE: dict = {}


def build_nc(per=PER, n=N, c=C, n_scaled_from=5, rows_per_part=50, bufs=8,
             reps=1, lwir_whole=True, store_eng="scalar", lwir_eng="gpsimd",
             sc_eng="gpsimd", alt_outputs=False, interleave_ls=False):
    """Build the single-core Bass program (SPMD: same program on all cores).

    reps>1 repeats the whole body (for benchmarking: amortizes dispatch
    noise); the op is idempotent so results are unchanged.
    """
    nc = bacc.Bacc()
    vis = nc.dram_tensor("vis", [per, n, c], F32, kind="ExternalInput")
    lwir = nc.dram_tensor("lwir", [per, n, c], F32, kind="ExternalInput")
    aware = nc.dram_tensor("aware", [per], F32, kind="ExternalInput")
    # Two outputs (visible half / lwir half of the anchor axis): keeps the
    # lwir passthrough DMAs free of WAW deps against the visible stores --
    # a single fused output tensor made neuronxcc reject the store DMAs
    # ("Too many sync wait commands"). The host gather concatenates.
    out_v = nc.dram_tensor("out_v", [per, n, c], F32, kind="ExternalOutput")
    out_l = nc.dram_tensor("out_l", [per, n, c], F32, kind="ExternalOutput")
    if alt_outputs:
        # bench-only: scratch second output set so odd reps have no WAW
        # deps against even reps (probes whether cross-rep WAW, not HBM,
        # limits the steady-state slope measurement)
        out_v2 = nc.dram_tensor("out_v2", [per, n, c], F32)
        out_l2 = nc.dram_tensor("out_l2", [per, n, c], F32)

    tile_rows = 128 * rows_per_part
    store_q = getattr(nc, store_eng)
    lwir_q = getattr(nc, lwir_eng)
    sc_q = getattr(nc, sc_eng)

    with TileContext(nc) as tc:
        with (
            tc.tile_pool(name="scales", bufs=1) as scpool,
            tc.tile_pool(name="data", bufs=bufs) as pool,
        ):
            sc = scpool.tile([128, per], F32)
            for b in range(per):
                src = aware[b : b + 1].rearrange("(r k) -> r k", r=1)
                sc_q.dma_start(out=sc[:, b : b + 1], in_=src.to_broadcast((128, 1)))

            for _rep in range(reps):
                if alt_outputs and _rep % 2 == 1:
                    o_v, o_l = out_v2, out_l2
                else:
                    o_v, o_l = out_v, out_l
                # lwir passthrough: o_l[b] = lwir[b]
                if lwir_whole:
                    lwir_q.dma_start(out=o_l[:, :, :], in_=lwir[:, :, :])
                else:
                    for b in range(per):
                        lwir_q.dma_start(out=o_l[b, :, :], in_=lwir[b, :, :])

                # visible: scale cols [n_scaled_from:] by s_b through SBUF
                t_idx = 0
                for b in range(per):
                    r = 0
                    while r < n:
                        rows = min(tile_rows, n - r)
                        assert rows % rows_per_part == 0
                        p = rows // rows_per_part
                        tile = pool.tile([p, rows_per_part, c], F32)
                        if interleave_ls:
                            load_q = nc.sync if t_idx % 2 == 0 else nc.scalar
                            st_q = nc.scalar if t_idx % 2 == 0 else nc.sync
                        else:
                            load_q, st_q = nc.sync, store_q
                        t_idx += 1
                        load_q.dma_start(
                            out=tile[:],
                            in_=vis[b, r : r + rows, :].rearrange(
                                "(p k) c -> p k c", p=p
                            ),
                        )
                        nc.vector.tensor_scalar(
                            tile[:, :, n_scaled_from:],
                            tile[:, :, n_scaled_from:],
                            sc[:p, b : b + 1],
                            None,
                            mybir.AluOpType.mult,
                        )
                        st_q.dma_start(
                            out=o_v[b, r : r + rows, :].rearrange(
                                "(p k) c -> p k c", p=p
                            ),
                            in_=tile[:],
                        )
                        r += rows
    nc.compile()
    return nc


def _get_nc():
    if "nc" not in _BUILD_CACHE:
        _BUILD_CACHE["nc"] = build_nc()
    return _BUILD_CACHE["nc"]


def run(inf_out_visible, inf_out_lwir, aware_score, trace=False, **kw):
    nc = _get_nc()
    # Pull everything to host numpy first: harness may hand us jax arrays,
    # and slicing those would dispatch XLA ops on the default (axon) backend.
    vis_np = np.asarray(inf_out_visible, dtype=np.float32)
    lwir_np = np.asarray(inf_out_lwir, dtype=np.float32)
    aw_np = np.asarray(aware_score, dtype=np.float32).reshape(B, -1)[:, 0]
    in_maps = []
    for core in range(NCORES):
        sl = slice(core * PER, (core + 1) * PER)
        in_maps.append(
            {
                "vis": np.ascontiguousarray(vis_np[sl]),
                "lwir": np.ascontiguousarray(lwir_np[sl]),
                "aware": np.ascontiguousarray(aw_np[sl]),
            }
        )
    try:
        res = run_bass_kernel_spmd(
            nc, in_maps, list(range(NCORES)), trace=trace, **kw
        )
    except Exception:
        # one retry: axon tunnel execute failures are transient and the
        # kernel is a pure function of its inputs
        res = run_bass_kernel_spmd(
            nc, in_maps, list(range(NCORES)), trace=trace, **kw
        )
    outs = [
        np.concatenate(
            [res.results[core]["out_v"], res.results[core]["out_l"]], axis=1
        )
        for core in range(NCORES)
    ]
    return np.concatenate(outs, axis=0), res


def kernel(inf_out_visible, inf_out_lwir, aware_score):
    out, _ = run(inf_out_visible, inf_out_lwir, aware_score)
    return out

